# revision 9
# baseline (speedup 1.0000x reference)
"""Trainium2 Bass kernel for nn_Block_38860864094289.

Sharding: data-parallel over batch B=32 across 8 NeuronCores (4 batches
per core), no collectives. Weights are replicated per core, shipped in
fp16. Inside each core:

  - LN1 -> h (fp16), transposed to hT [c, t] via PE transposes
  - qT/kT = w-stationary matmuls; v per batch in normal layout
  - relu-sum attention scores: T[c, i, j] = q[c,i] + k[c,j] built with a
    broadcast scalar_tensor_tensor on DVE, relu split DVE/ACT, then
    reduced over c (partitions) with a ones-vector matmul on the PE into
    per-i-group PSUM strips, scattered to score[i, j] by DMA
  - softmax (fp32) with exp+accum on ACT; attn @ v and proj via PE
  - branch 1 (rows 0:40): LN2 + MLP (gelu exact) with residual
  - branch 2 (rows 40:81): the whole DWT/bilinear/IDWT pipeline is
    folded into host-precomputed composite matrices:
      low = Pf @ hf @ Pc ; u = gelu(low@w21+b21)@w22+b22
      rec = F0 @ u @ Gc + F1 @ hf @ N1 + F2 @ hf @ N2 ; x2 = xb + rec
"""

import numpy as np

B, F, C, HID = 32, 81, 512, 2048
NCORES = 8
BL = B // NCORES  # batches per core
F1H = F // 2      # 40 rows, branch 1
F2H = F - F1H     # 41 rows, branch 2
_S = 1.0 / np.sqrt(2.0)

_CACHE = {}


# --------------------------------------------------------------------------
# host-side composite matrices for the wavelet branch (float64 -> fp16/fp32)
def _up_mat(h, H):
    ys = np.linspace(0.0, h - 1.0, H)
    y0 = np.floor(ys).astype(int)
    y1 = np.minimum(y0 + 1, h - 1)
    wy = ys - y0
    R = np.zeros((H, h))
    for i in range(H):
        R[i, y0[i]] += 1.0 - wy[i]
        R[i, y1[i]] += wy[i]
    return R


def host_mats():
    Wa = np.zeros((C, 256))
    Wd = np.zeros((C, 256))
    for m in range(256):
        Wa[2 * m, m] = _S
        Wa[2 * m + 1, m] = _S
        Wd[2 * m, m] = _S
        Wd[2 * m + 1, m] = -_S
    Ha = np.zeros((21, F2H))
    Hd = np.zeros((21, F2H))
    for m in range(21):
        r0, r1 = 2 * m, 2 * m + 1
        Ha[m, r0] = _S
        Hd[m, r0] = _S
        if r1 < F2H:
            Ha[m, r1] = _S
            Hd[m, r1] = -_S
    Rh = _up_mat(21, F2H)
    Rw = _up_mat(256, C)
    Sh = _up_mat(F2H, 21)
    Sw = _up_mat(C, 256)
    A = np.zeros((42, 21))
    Bf = np.zeros((42, 21))
    for m in range(21):
        A[2 * m, m] = _S
        A[2 * m + 1, m] = _S
        Bf[2 * m, m] = _S
        Bf[2 * m + 1, m] = -_S
    Ce = np.zeros((256, C))
    Cd = np.zeros((256, C))
    for m in range(256):
        Ce[m, 2 * m] = _S
        Ce[m, 2 * m + 1] = _S
        Cd[m, 2 * m] = _S
        Cd[m, 2 * m + 1] = -_S
    Pf = Rh @ Ha                 # [41, 41]
    Pc = Wa @ Rw.T               # [512, 512]
    F0 = (A @ Sh)[:F2H]          # [41, 41]
    Gc = Sw.T @ Ce               # [512, 512]
    F1m = (Bf @ Hd)[:F2H]
    N1 = Wa @ Ce + Wd @ Cd
    F2m = (A @ Ha)[:F2H]
    N2 = Wd @ Cd
    return dict(
        pft=Pf.T.astype(np.float16), f0t=F0.T.astype(np.float16),
        f1t=F1m.T.astype(np.float16), f2t=F2m.T.astype(np.float16),
        pc=Pc.astype(np.float16), gc=Gc.astype(np.float16),
        n1m=N1.astype(np.float16), n2m=N2.astype(np.float16),
    )


# --------------------------------------------------------------------------
def build_nc():
    import concourse.bass as bass
    import concourse.bacc as bacc
    import concourse.mybir as mybir
    import concourse.tile as tile
    from concourse.masks import make_identity

    F16 = mybir.dt.float16
    F32 = mybir.dt.float32
    AL = mybir.AluOpType
    AF = mybir.ActivationFunctionType
    AX = mybir.AxisListType

    nc = bacc.Bacc("TRN2", target_bir_lowering=False, debug=False,
                   num_devices=NCORES)

    # ---------------- DRAM I/O -------------------------------------------
    xc = nc.dram_tensor("xc", [BL, F, C], F32, kind="ExternalInput")
    yc = nc.dram_tensor("yc", [BL, F, C], F32, kind="ExternalOutput")
    dr = {}
    for nm in ("wq", "wk", "wv", "wp"):
        dr[nm] = nc.dram_tensor(nm, [C, C], F16, kind="ExternalInput")
    for nm in ("pc", "gc", "n1m", "n2m"):
        dr[nm] = nc.dram_tensor(nm, [C, C], F16, kind="ExternalInput")
    for nm in ("w11", "w21"):
        dr[nm] = nc.dram_tensor(nm, [C, HID], F16, kind="ExternalInput")
    for nm in ("w12", "w22"):
        dr[nm] = nc.dram_tensor(nm, [HID, C], F16, kind="ExternalInput")
    for nm in ("b11", "b21"):
        dr[nm] = nc.dram_tensor(nm, [HID], F32, kind="ExternalInput")
    for nm in ("bp", "b12", "b22", "n1w", "n1b", "n2w", "n2b", "n3w", "n3b"):
        dr[nm] = nc.dram_tensor(nm, [C], F32, kind="ExternalInput")
    for nm in ("pft", "f0t", "f1t", "f2t"):
        dr[nm] = nc.dram_tensor(nm, [F2H, F2H], F16, kind="ExternalInput")

    def bcast_ap(t, n):
        return bass.AP(tensor=t, offset=0, ap=[[0, 128], [1, n]])

    from contextlib import ExitStack
    with tile.TileContext(nc) as tc, ExitStack() as stack:
        wp_ = stack.enter_context(tc.tile_pool(name="w", bufs=1))
        ap_ = stack.enter_context(tc.tile_pool(name="a", bufs=1))
        lp = stack.enter_context(tc.tile_pool(name="lp", bufs=2))

        # ------------- weights / constants to SBUF ------------------------
        wsb = {}
        for nm in ("wq", "wk", "wv", "wp", "pc", "gc", "n1m", "n2m"):
            t = wp_.tile([128, 4, C], F16, tag=nm)
            nc.sync.dma_start(out=t[:, :, :],
                              in_=dr[nm].rearrange("(k p) o -> p k o", p=128))
            wsb[nm] = t
        # w11/w21 share slots, w12/w22 share slots (sequential use)
        w11 = wp_.tile([128, 4, HID], F16, tag="wA")
        nc.sync.dma_start(out=w11[:, :, :],
                          in_=dr["w11"].rearrange("(k p) o -> p k o", p=128))
        w12 = wp_.tile([128, 16, C], F16, tag="wB")
        nc.sync.dma_start(out=w12[:, :, :],
                          in_=dr["w12"].rearrange("(k p) o -> p k o", p=128))
        b11 = wp_.tile([128, 16], F32, tag="b11")
        nc.sync.dma_start(out=b11[:, :],
                          in_=dr["b11"].rearrange("(k p) -> p k", p=128))
        b21 = wp_.tile([128, 16], F32, tag="b21")
        nc.sync.dma_start(out=b21[:, :],
                          in_=dr["b21"].rearrange("(k p) -> p k", p=128))
        b22pp = wp_.tile([128, 4], F32, tag="b22pp")
        nc.sync.dma_start(out=b22pp[:, :],
                          in_=dr["b22"].rearrange("(k p) -> p k", p=128))
        bc = {}
        for nm in ("bp", "b12", "n1w", "n1b", "n2w", "n2b", "n3w", "n3b"):
            t = wp_.tile([128, C], F32, tag="bc" + nm)
            nc.sync.dma_start(out=t[:, :], in_=bcast_ap(dr[nm], C))
            bc[nm] = t
        fm = {}
        for nm in ("pft", "f0t", "f1t", "f2t"):
            t = wp_.tile([F2H, F2H], F16, tag=nm)
            nc.sync.dma_start(out=t[:, :], in_=dr[nm][:, :])
            fm[nm] = t
        ident = wp_.tile([128, 128], F16, tag="ident")
        make_identity(nc, ident[:, :])
        ones = wp_.tile([128, 1], F16, tag="ones")
        nc.vector.memset(ones[:, :], 1.0)
        eps = wp_.tile([128, 1], F32, tag="eps")
        nc.vector.memset(eps[:, :], 1e-5)

        # ------------- persistent activation tiles ------------------------
        TOK = BL * F           # 324
        hT = ap_.tile([128, 4, TOK], F16, tag="hT")
        qT = ap_.tile([128, 4, TOK], F16, tag="qT")
        kT = ap_.tile([128, 4, TOK], F16, tag="kT")
        x_b = [ap_.tile([F, C], F32, tag=f"x{b}", name=f"x_b{b}") for b in range(BL)]
        v_b = [ap_.tile([F, C], F16, tag=f"v{b}", name=f"v_b{b}") for b in range(BL)]

        def layer_norm(src, rows, gname, bname, dst):
            """src [rows, C] f32 (partitions 0..rows-1) -> dst [rows, C] f16"""
            st = lp.tile([128, 6], F32, tag="ln_st")
            mv = lp.tile([128, 2], F32, tag="ln_mv")
            rstd = lp.tile([128, 1], F32, tag="ln_rstd")
            xh = lp.tile([128, C], F32, tag="ln_xh")
            nc.vector.bn_stats(out=st[:rows, :], in_=src)
            nc.vector.bn_aggr(out=mv[:rows, :], in_=st[:rows, :])
            nc.scalar.activation(out=rstd[:rows, :], in_=mv[:rows, 1:2],
                                 func=AF.Sqrt, bias=eps[:rows, :], scale=1.0)
            nc.vector.reciprocal(out=rstd[:rows, :], in_=rstd[:rows, :])
            nc.vector.tensor_scalar(out=xh[:rows, :], in0=src,
                                    scalar1=mv[:rows, 0:1],
                                    scalar2=rstd[:rows, :],
                                    op0=AL.subtract, op1=AL.mult)
            nc.vector.tensor_mul(out=xh[:rows, :], in0=xh[:rows, :],
                                 in1=bc[gname][:rows, :])
            nc.vector.tensor_add(out=dst, in0=xh[:rows, :],
                                 in1=bc[bname][:rows, :])

        # ================= stage A: LN1, h, hT ============================
        ppAB = tc.tile_pool(name="ppAB", bufs=2, space="PSUM")
        pp = ppAB.__enter__()
        for b in range(BL):
            nc.sync.dma_start(out=x_b[b][:, :], in_=xc[b])
            h_b = lp.tile([F, C], F16, tag="h_b")
            layer_norm(x_b[b][:, :], F, "n1w", "n1b", h_b[:, :])
            for ch in range(4):
                ps = pp.tile([128, F], F16, tag="tpA")
                nc.tensor.transpose(out=ps[:, :],
                                    in_=h_b[:, ch * 128:(ch + 1) * 128],
                                    identity=ident[0:F, 0:F])
                nc.vector.tensor_copy(out=hT[:, ch, b * F:(b + 1) * F],
                                      in_=ps[:, :])

        # ================= stage B: qT, kT, v =============================
        for nm, dst in (("wq", qT), ("wk", kT)):
            for ch in range(4):
                ps = pp.tile([128, TOK], F32, tag="psQK")
                for kk in range(4):
                    nc.tensor.matmul(out=ps[:, :],
                                     lhsT=wsb[nm][:, kk, ch * 128:(ch + 1) * 128],
                                     rhs=hT[:, kk, :],
                                     start=(kk == 0), stop=(kk == 3))
                nc.vector.tensor_copy(out=dst[:, ch, :], in_=ps[:, :])
        for b in range(BL):
            ps = pp.tile([F, C], F32, tag="psV")
            for kk in range(4):
                nc.tensor.matmul(out=ps[:, :],
                                 lhsT=hT[:, kk, b * F:(b + 1) * F],
                                 rhs=wsb["wv"][:, kk, :],
                                 start=(kk == 0), stop=(kk == 3))
            nc.vector.tensor_copy(out=v_b[b][:, :], in_=ps[:, :])

        ppAB.__exit__(None, None, None)
        # ================= stage C: attention =============================
        ppC = tc.tile_pool(name="ppC", bufs=2, space="PSUM")
        pp = ppC.__enter__()
        NG = 9           # i-group size; 81 = 9 groups of 9
        NT = NG * F      # 729 strip length
        for b in range(BL):
            scoreM = lp.tile([F, F], F32, tag="scoreM")
            for g in range(9):
                i0 = g * NG
                Tb = lp.tile([128, 4, NG, F], F16, tag="Tb")
                for ch in range(4):
                    ks = kT[:, ch, b * F:(b + 1) * F]
                    kb = bass.AP(tensor=ks.tensor, offset=ks.offset,
                                 ap=[ks.ap[0], [0, NG], ks.ap[-1]])
                    qs = qT[:, ch, b * F + i0:b * F + i0 + NG]
                    qb = bass.AP(tensor=qs.tensor, offset=qs.offset,
                                 ap=[qs.ap[0], qs.ap[-1], [0, F]])
                    nc.vector.scalar_tensor_tensor(
                        out=Tb[:, ch, :, :], in0=kb, scalar=0.0, in1=qb,
                        op0=AL.add, op1=AL.add)
                # relu: chunk 0 on DVE, chunks 1-3 on ACT
                nc.vector.tensor_scalar(out=Tb[:, 0, :, :],
                                        in0=Tb[:, 0, :, :],
                                        scalar1=0.0, scalar2=None,
                                        op0=AL.max)
                nc.scalar.activation(out=Tb[:, 1:4, :, :],
                                     in_=Tb[:, 1:4, :, :], func=AF.Relu)
                ps = pp.tile([1, NT], F32, tag="psS")
                for (o, n) in ((0, 512), (512, NT - 512)):
                    for ch in range(4):
                        nc.tensor.matmul(
                            out=ps[0:1, o:o + n],
                            lhsT=ones[:, :],
                            rhs=Tb[:, ch].rearrange("p a b -> p (a b)")[:, o:o + n],
                            start=(ch == 0), stop=(ch == 3))
                sc = lp.tile([1, NT], F32, tag="sc")
                if g % 2 == 0:
                    nc.vector.tensor_copy(out=sc[:, :], in_=ps[0:1, :])
                else:
                    nc.scalar.copy(out=sc[:, :], in_=ps[0:1, :])
                nc.sync.dma_start(out=scoreM[i0:i0 + NG, :], in_=sc[:, :])
            # softmax
            mrow = lp.tile([F, 1], F32, tag="mrow")
            nmrow = lp.tile([F, 1], F32, tag="nmrow")
            sexp = lp.tile([F, 1], F32, tag="sexp")
            attn = lp.tile([F, F], F16, tag="attn")
            attnT = lp.tile([F, F], F16, tag="attnT")
            nc.vector.reduce_max(out=mrow[:, :], in_=scoreM[:, :], axis=AX.X)
            nc.vector.tensor_scalar(out=nmrow[:, :], in0=mrow[:, :],
                                    scalar1=-1.0, scalar2=None, op0=AL.mult)
            nc.scalar.activation(out=attn[:, :], in_=scoreM[:, :], func=AF.Exp,
                                 bias=nmrow[:, :], scale=1.0,
                                 accum_out=sexp[:, :])
            nc.vector.reciprocal(out=sexp[:, :], in_=sexp[:, :])
            nc.vector.tensor_scalar(out=attn[:, :], in0=attn[:, :],
                                    scalar1=sexp[:, :], scalar2=None,
                                    op0=AL.mult)
            psT = pp.tile([F, F], F16, tag="c_small", bufs=3, name="psT")
            nc.tensor.transpose(out=psT[:, :], in_=attn[:, :],
                                identity=ident[0:F, 0:F])
            nc.vector.tensor_copy(out=attnT[:, :], in_=psT[:, :])
            aTs = lp.tile([128, 4, F], F16, tag="aTs")
            for ch in range(4):
                psV = pp.tile([128, F], F32, tag="c_small", bufs=3, name="psV")
                nc.tensor.matmul(out=psV[:, :],
                                 lhsT=v_b[b][:, ch * 128:(ch + 1) * 128],
                                 rhs=attnT[:, :], start=True, stop=True)
                nc.vector.tensor_copy(out=aTs[:, ch, :], in_=psV[:, :])
            psP = pp.tile([F, C], F32, tag="c_small", bufs=3, name="psP")
            for ch in range(4):
                nc.tensor.matmul(out=psP[:, :], lhsT=aTs[:, ch, :],
                                 rhs=wsb["wp"][:, ch, :],
                                 start=(ch == 0), stop=(ch == 3))
            nc.vector.scalar_tensor_tensor(out=x_b[b][:, :], in0=psP[:, :],
                                           scalar=0.0, in1=x_b[b][:, :],
                                           op0=AL.bypass, op1=AL.add)
            nc.vector.tensor_add(out=x_b[b][:, :], in0=x_b[b][:, :],
                                 in1=bc["bp"][0:F, :])

        ppC.__exit__(None, None, None)
        # ================= stage D: branch 1 (rows 0:40) ==================
        ppD = tc.tile_pool(name="ppD", bufs=2, space="PSUM")
        pp = ppD.__enter__()
        h2T = ap_.tile([128, 4, BL * F1H], F16, tag="h2T")
        for b in range(BL):
            h2_b = lp.tile([F1H, C], F16, tag="h2_b")
            layer_norm(x_b[b][0:F1H, :], F1H, "n2w", "n2b", h2_b[:, :])
            for ch in range(4):
                ps = pp.tile([128, F1H], F16, tag="tpD")
                nc.tensor.transpose(out=ps[:, :],
                                    in_=h2_b[:, ch * 128:(ch + 1) * 128],
                                    identity=ident[0:F1H, 0:F1H])
                nc.vector.tensor_copy(
                    out=h2T[:, ch, b * F1H:(b + 1) * F1H], in_=ps[:, :])
        xa_p0 = ap_.tile([128, C], F32, tag="xa_p0")
        xa_p1 = ap_.tile([32, C], F32, tag="xa_p1")
        for b in range(BL):
            r0 = b * F1H
            if r0 + F1H <= 128:
                nc.sync.dma_start(out=xa_p0[r0:r0 + F1H, :],
                                  in_=x_b[b][0:F1H, :])
            else:
                nc.sync.dma_start(out=xa_p0[r0:128, :],
                                  in_=x_b[b][0:128 - r0, :])
                nc.sync.dma_start(out=xa_p1[0:r0 + F1H - 128, :],
                                  in_=x_b[b][128 - r0:F1H, :])
        g_sb = ap_.tile([128, 16, BL * F2H], F16, tag="g")  # sized for max use
        TOK1 = BL * F1H
        for hch in range(16):
            ps = pp.tile([128, TOK1], F32, tag="psF1")
            for kk in range(4):
                nc.tensor.matmul(out=ps[:, :],
                                 lhsT=w11[:, kk, hch * 128:(hch + 1) * 128],
                                 rhs=h2T[:, kk, :],
                                 start=(kk == 0), stop=(kk == 3))
            nc.scalar.activation(out=g_sb[:, hch, 0:TOK1], in_=ps[:, :],
                                 func=AF.Gelu, bias=b11[:, hch:hch + 1],
                                 scale=1.0)
        for (tsl, tn) in ((0, 128), (128, 32)):
            ps = pp.tile([128, C], F32, tag="psF2")
            for hch in range(16):
                nc.tensor.matmul(out=ps[0:tn, :],
                                 lhsT=g_sb[:, hch, tsl:tsl + tn],
                                 rhs=w12[:, hch, :],
                                 start=(hch == 0), stop=(hch == 15))
            o1 = lp.tile([128, C], F32, tag="o1")
            xa_src = xa_p0[tsl:tsl + tn, :] if tsl == 0 else xa_p1[0:tn, :]
            nc.vector.scalar_tensor_tensor(out=o1[0:tn, :], in0=ps[0:tn, :],
                                           scalar=0.0, in1=xa_src,
                                           op0=AL.bypass, op1=AL.add)
            nc.vector.tensor_add(out=o1[0:tn, :], in0=o1[0:tn, :],
                                 in1=bc["b12"][0:tn, :])
            for b in range(BL):
                r0, r1 = b * F1H, (b + 1) * F1H
                lo, hi = max(r0, tsl), min(r1, tsl + tn)
                if lo < hi:
                    nc.sync.dma_start(
                        out=yc[b, lo - r0:hi - r0, :],
                        in_=o1[lo - tsl:hi - tsl, :])

        ppD.__exit__(None, None, None)
        # ================= stage E: branch 2 (rows 40:81) =================
        ppE = tc.tile_pool(name="ppE", bufs=2, space="PSUM")
        pp = ppE.__enter__()
        # load branch-2 MLP weights into the shared slots
        w21 = wp_.tile([128, 4, HID], F16, tag="wA")
        nc.sync.dma_start(out=w21[:, :, :],
                          in_=dr["w21"].rearrange("(k p) o -> p k o", p=128))
        w22 = wp_.tile([128, 16, C], F16, tag="wB")
        nc.sync.dma_start(out=w22[:, :, :],
                          in_=dr["w22"].rearrange("(k p) o -> p k o", p=128))
        TOK2 = BL * F2H  # 164
        hfT = ap_.tile([128, 4, TOK2], F16, tag="hfT")
        psiT = ap_.tile([128, 4, TOK2], F16, tag="psiT")
        zT = ap_.tile([128, 4, TOK2], F16, tag="zT")
        wT = ap_.tile([128, 4, TOK2], F16, tag="wT")
        xb0 = [ap_.tile([F2H, C], F32, tag=f"xb0_{b}", name=f"xb0_{b}") for b in range(BL)]
        for b in range(BL):
            nc.sync.dma_start(out=xb0[b][:, :], in_=x_b[b][F1H:F, :])
            hf_b = lp.tile([F2H, C], F16, tag="hf_b")
            layer_norm(xb0[b][:, :], F2H, "n3w", "n3b", hf_b[:, :])
            for ch in range(4):
                ps = pp.tile([128, F2H], F16, tag="e_tp", bufs=2, name="psTpE")
                nc.tensor.transpose(out=ps[:, :],
                                    in_=hf_b[:, ch * 128:(ch + 1) * 128],
                                    identity=ident[0:F2H, 0:F2H])
                nc.vector.tensor_copy(
                    out=hfT[:, ch, b * F2H:(b + 1) * F2H], in_=ps[:, :])
            for ch in range(4):
                for nm, dst in (("pft", psiT), ("f1t", zT), ("f2t", wT)):
                    ps = pp.tile([128, TOK2], F32, tag="e_mm", bufs=3, name="psZ")
                    nc.tensor.matmul(out=ps[:, 0:F2H],
                                     lhsT=hf_b[:, ch * 128:(ch + 1) * 128],
                                     rhs=fm[nm][:, :], start=True, stop=True)
                    nc.vector.tensor_copy(
                        out=dst[:, ch, b * F2H:(b + 1) * F2H], in_=ps[:, 0:F2H])
        lowT = ap_.tile([128, 4, TOK2], F16, tag="lowT")
        for ch in range(4):
            ps = pp.tile([128, TOK2], F32, tag="e_mm", bufs=3, name="psL")
            for kk in range(4):
                nc.tensor.matmul(out=ps[:, :],
                                 lhsT=wsb["pc"][:, kk, ch * 128:(ch + 1) * 128],
                                 rhs=psiT[:, kk, :],
                                 start=(kk == 0), stop=(kk == 3))
            nc.vector.tensor_copy(out=lowT[:, ch, :], in_=ps[:, :])
        for hch in range(16):
            ps = pp.tile([128, TOK2], F32, tag="e_mm", bufs=3, name="psF3")
            for kk in range(4):
                nc.tensor.matmul(out=ps[:, :],
                                 lhsT=w21[:, kk, hch * 128:(hch + 1) * 128],
                                 rhs=lowT[:, kk, :],
                                 start=(kk == 0), stop=(kk == 3))
            nc.scalar.activation(out=g_sb[:, hch, 0:TOK2], in_=ps[:, :],
                                 func=AF.Gelu, bias=b21[:, hch:hch + 1],
                                 scale=1.0)
        uT = ap_.tile([128, 4, TOK2], F16, tag="uT")
        for och in range(4):
            ps = pp.tile([128, TOK2], F32, tag="e_mm", bufs=3, name="psF4")
            for hch in range(16):
                nc.tensor.matmul(out=ps[:, :],
                                 lhsT=w22[:, hch, och * 128:(och + 1) * 128],
                                 rhs=g_sb[:, hch, 0:TOK2],
                                 start=(hch == 0), stop=(hch == 15))
            nc.scalar.activation(out=uT[:, och, :], in_=ps[:, :],
                                 func=AF.Identity,
                                 bias=b22pp[:, och:och + 1], scale=1.0)
        t1T = ap_.tile([128, 4, TOK2], F16, tag="t1T")
        for o2 in range(4):
            ps = pp.tile([128, TOK2], F32, tag="e_mm", bufs=3, name="psT1")
            for och in range(4):
                nc.tensor.matmul(out=ps[:, :],
                                 lhsT=wsb["gc"][:, och, o2 * 128:(o2 + 1) * 128],
                                 rhs=uT[:, och, :],
                                 start=(och == 0), stop=(och == 3))
            nc.vector.tensor_copy(out=t1T[:, o2, :], in_=ps[:, :])
        for b in range(BL):
            t1_b = lp.tile([F2H, 4, 128], F16, tag="t1_b")
            for ch in range(4):
                ps = pp.tile([F2H, 128], F16, tag="e_tp", bufs=2, name="psTpT1")
                nc.tensor.transpose(out=ps[:, :],
                                    in_=t1T[:, ch, b * F2H:(b + 1) * F2H],
                                    identity=ident[:, :])
                nc.vector.tensor_copy(out=t1_b[:, ch, :], in_=ps[:, :])
            psR = pp.tile([F2H, C], F32, tag="psR", bufs=2)
            nc.tensor.matmul(out=psR[:, :], lhsT=fm["f0t"][:, :],
                             rhs=t1_b.rearrange("p a b -> p (a b)"),
                             start=True, stop=False)
            for ch in range(4):
                nc.tensor.matmul(out=psR[:, :],
                                 lhsT=zT[:, ch, b * F2H:(b + 1) * F2H],
                                 rhs=wsb["n1m"][:, ch, :],
                                 start=False, stop=False)
            for ch in range(4):
                nc.tensor.matmul(out=psR[:, :],
                                 lhsT=wT[:, ch, b * F2H:(b + 1) * F2H],
                                 rhs=wsb["n2m"][:, ch, :],
                                 start=False, stop=(ch == 3))
            nc.vector.scalar_tensor_tensor(out=xb0[b][:, :], in0=psR[:, :],
                                           scalar=0.0, in1=xb0[b][:, :],
                                           op0=AL.bypass, op1=AL.add)
            nc.sync.dma_start(out=yc[b, F1H:F, :], in_=xb0[b][:, :])
        ppE.__exit__(None, None, None)

    nc.compile()
    return nc


def _prep_inputs(inputs):
    """Full inputs -> (common weight map, list of per-core maps)."""
    ins = {k: np.asarray(v) for k, v in inputs.items()}
    mats = host_mats()
    com = {}
    for nm in ("wq", "wk", "wv", "wp", "w11", "w12", "w21", "w22"):
        com[nm] = ins[nm].astype(np.float16)
    for nm, arr in mats.items():
        com[nm] = arr
    for src, dst in (("bp", "bp"), ("b11", "b11"), ("b12", "b12"),
                     ("b21", "b21"), ("b22", "b22"),
                     ("norm1_w", "n1w"), ("norm1_b", "n1b"),
                     ("norm2_w", "n2w"), ("norm2_b", "n2b"),
                     ("norm3_w", "n3w"), ("norm3_b", "n3b")):
        com[dst] = ins[src].astype(np.float32)
    x = ins["x"].astype(np.float32)
    in_maps = []
    for c in range(NCORES):
        m = dict(com)
        m["xc"] = np.ascontiguousarray(x[c * BL:(c + 1) * BL])
        in_maps.append(m)
    return in_maps


def kernel(**inputs):
    from concourse.bass_utils import run_bass_kernel_spmd
    if "nc" not in _CACHE:
        _CACHE["nc"] = build_nc()
    nc = _CACHE["nc"]
    in_maps = _prep_inputs(inputs)
    res = run_bass_kernel_spmd(nc, in_maps, core_ids=list(range(NCORES)))
    out = np.concatenate([res.results[c]["yc"] for c in range(NCORES)], axis=0)
    return out.astype(np.float32)


# revision 10
# speedup vs baseline: 7248.5275x; 7248.5275x over previous
"""Trainium2 Bass kernel for nn_Block_38860864094289.

Sharding: data-parallel over batch B=32 across 8 NeuronCores (4 batches
per core), no collectives. Weights are replicated per core, shipped in
fp16. Inside each core:

  - LN1 -> h (fp16), transposed to hT [c, t] via PE transposes
  - qT/kT = w-stationary matmuls; v per batch in normal layout
  - relu-sum attention scores: T[c, i, j] = q[c,i] + k[c,j] built with a
    broadcast scalar_tensor_tensor on DVE, relu split DVE/ACT, then
    reduced over c (partitions) with a ones-vector matmul on the PE into
    per-i-group PSUM strips, scattered to score[i, j] by DMA
  - softmax (fp32) with exp+accum on ACT; attn @ v and proj via PE
  - branch 1 (rows 0:40): LN2 + MLP (gelu exact) with residual
  - branch 2 (rows 40:81): the whole DWT/bilinear/IDWT pipeline is
    folded into host-precomputed composite matrices:
      low = Pf @ hf @ Pc ; u = gelu(low@w21+b21)@w22+b22
      rec = F0 @ u @ Gc + F1 @ hf @ N1 + F2 @ hf @ N2 ; x2 = xb + rec
"""

import numpy as np

B, F, C, HID = 32, 81, 512, 2048
NCORES = 8
BL = B // NCORES  # batches per core
F1H = F // 2      # 40 rows, branch 1
F2H = F - F1H     # 41 rows, branch 2
_S = 1.0 / np.sqrt(2.0)

_CACHE = {}


# --------------------------------------------------------------------------
# host-side composite matrices for the wavelet branch (float64 -> fp16/fp32)
def _up_mat(h, H):
    ys = np.linspace(0.0, h - 1.0, H)
    y0 = np.floor(ys).astype(int)
    y1 = np.minimum(y0 + 1, h - 1)
    wy = ys - y0
    R = np.zeros((H, h))
    for i in range(H):
        R[i, y0[i]] += 1.0 - wy[i]
        R[i, y1[i]] += wy[i]
    return R


def host_mats():
    Wa = np.zeros((C, 256))
    Wd = np.zeros((C, 256))
    for m in range(256):
        Wa[2 * m, m] = _S
        Wa[2 * m + 1, m] = _S
        Wd[2 * m, m] = _S
        Wd[2 * m + 1, m] = -_S
    Ha = np.zeros((21, F2H))
    Hd = np.zeros((21, F2H))
    for m in range(21):
        r0, r1 = 2 * m, 2 * m + 1
        Ha[m, r0] = _S
        Hd[m, r0] = _S
        if r1 < F2H:
            Ha[m, r1] = _S
            Hd[m, r1] = -_S
    Rh = _up_mat(21, F2H)
    Rw = _up_mat(256, C)
    Sh = _up_mat(F2H, 21)
    Sw = _up_mat(C, 256)
    A = np.zeros((42, 21))
    Bf = np.zeros((42, 21))
    for m in range(21):
        A[2 * m, m] = _S
        A[2 * m + 1, m] = _S
        Bf[2 * m, m] = _S
        Bf[2 * m + 1, m] = -_S
    Ce = np.zeros((256, C))
    Cd = np.zeros((256, C))
    for m in range(256):
        Ce[m, 2 * m] = _S
        Ce[m, 2 * m + 1] = _S
        Cd[m, 2 * m] = _S
        Cd[m, 2 * m + 1] = -_S
    Pf = Rh @ Ha                 # [41, 41]
    Pc = Wa @ Rw.T               # [512, 512]
    F0 = (A @ Sh)[:F2H]          # [41, 41]
    Gc = Sw.T @ Ce               # [512, 512]
    F1m = (Bf @ Hd)[:F2H]
    N1 = Wa @ Ce + Wd @ Cd
    F2m = (A @ Ha)[:F2H]
    N2 = Wd @ Cd
    return dict(
        pft=Pf.T.astype(np.float16), f0t=F0.T.astype(np.float16),
        f1t=F1m.T.astype(np.float16), f2t=F2m.T.astype(np.float16),
        pc=Pc.astype(np.float16), gc=Gc.astype(np.float16),
        n1m=N1.astype(np.float16), n2m=N2.astype(np.float16),
    )


# --------------------------------------------------------------------------
def build_nc(iters=1, share_w=True):
    import concourse.bass as bass
    import concourse.bacc as bacc
    import concourse.mybir as mybir
    import concourse.tile as tile
    from concourse.masks import make_identity

    F16 = mybir.dt.float16
    F32 = mybir.dt.float32
    AL = mybir.AluOpType
    AF = mybir.ActivationFunctionType
    AX = mybir.AxisListType

    nc = bacc.Bacc("TRN2", target_bir_lowering=False, debug=False,
                   num_devices=NCORES)

    # ---------------- DRAM I/O -------------------------------------------
    xc = nc.dram_tensor("xc", [BL, F, C], F32, kind="ExternalInput")
    yc = nc.dram_tensor("yc", [BL, F, C], F32, kind="ExternalOutput")
    dr = {}
    for nm in ("wq", "wk", "wv", "wp"):
        dr[nm] = nc.dram_tensor(nm, [C, C], F16, kind="ExternalInput")
    for nm in ("pc", "gc", "n1m", "n2m"):
        dr[nm] = nc.dram_tensor(nm, [C, C], F16, kind="ExternalInput")
    for nm in ("w11", "w21"):
        dr[nm] = nc.dram_tensor(nm, [C, HID], F16, kind="ExternalInput")
    for nm in ("w12", "w22"):
        dr[nm] = nc.dram_tensor(nm, [HID, C], F16, kind="ExternalInput")
    for nm in ("b11", "b21"):
        dr[nm] = nc.dram_tensor(nm, [HID], F32, kind="ExternalInput")
    for nm in ("bp", "b12", "b22", "n1w", "n1b", "n2w", "n2b", "n3w", "n3b"):
        dr[nm] = nc.dram_tensor(nm, [C], F32, kind="ExternalInput")
    for nm in ("pft", "f0t", "f1t", "f2t"):
        dr[nm] = nc.dram_tensor(nm, [F2H, F2H], F16, kind="ExternalInput")

    def bcast_ap(t, n):
        return bass.AP(tensor=t, offset=0, ap=[[0, 128], [1, n]])

    from contextlib import ExitStack
    with tile.TileContext(nc) as tc, ExitStack() as stack:
        wp_ = stack.enter_context(tc.tile_pool(name="w", bufs=1))
        ap_ = stack.enter_context(tc.tile_pool(name="a", bufs=1))
        lp = stack.enter_context(tc.tile_pool(name="lp", bufs=2))

        # ------------- weights / constants to SBUF ------------------------
        wsb = {}
        for nm in ("wq", "wk", "wv", "wp", "pc", "gc", "n1m", "n2m"):
            t = wp_.tile([128, 4, C], F16, tag=nm)
            nc.sync.dma_start(out=t[:, :, :],
                              in_=dr[nm].rearrange("(k p) o -> p k o", p=128))
            wsb[nm] = t
        # w11/w21 share slots, w12/w22 share slots (sequential use)
        w11 = wp_.tile([128, 4, HID], F16, tag="wA")
        nc.sync.dma_start(out=w11[:, :, :],
                          in_=dr["w11"].rearrange("(k p) o -> p k o", p=128))
        w12 = wp_.tile([128, 16, C], F16, tag="wB")
        nc.sync.dma_start(out=w12[:, :, :],
                          in_=dr["w12"].rearrange("(k p) o -> p k o", p=128))
        b11 = wp_.tile([128, 16], F32, tag="b11")
        nc.sync.dma_start(out=b11[:, :],
                          in_=dr["b11"].rearrange("(k p) -> p k", p=128))
        b21 = wp_.tile([128, 16], F32, tag="b21")
        nc.sync.dma_start(out=b21[:, :],
                          in_=dr["b21"].rearrange("(k p) -> p k", p=128))
        b22pp = wp_.tile([128, 4], F32, tag="b22pp")
        nc.sync.dma_start(out=b22pp[:, :],
                          in_=dr["b22"].rearrange("(k p) -> p k", p=128))
        bc = {}
        for nm in ("bp", "b12", "n1w", "n1b", "n2w", "n2b", "n3w", "n3b"):
            t = wp_.tile([128, C], F32, tag="bc" + nm)
            nc.sync.dma_start(out=t[:, :], in_=bcast_ap(dr[nm], C))
            bc[nm] = t
        fm = {}
        for nm in ("pft", "f0t", "f1t", "f2t"):
            t = wp_.tile([F2H, F2H], F16, tag=nm)
            nc.sync.dma_start(out=t[:, :], in_=dr[nm][:, :])
            fm[nm] = t
        w21_pre = w22_pre = None
        if not share_w:
            w21_pre = wp_.tile([128, 4, HID], F16, tag="wA2", name="w21p")
            nc.sync.dma_start(out=w21_pre[:, :, :],
                              in_=dr["w21"].rearrange("(k p) o -> p k o", p=128))
            w22_pre = wp_.tile([128, 16, C], F16, tag="wB2", name="w22p")
            nc.sync.dma_start(out=w22_pre[:, :, :],
                              in_=dr["w22"].rearrange("(k p) o -> p k o", p=128))
        ident = wp_.tile([128, 128], F16, tag="ident")
        make_identity(nc, ident[:, :])
        ones = wp_.tile([128, 1], F16, tag="ones")
        nc.vector.memset(ones[:, :], 1.0)
        eps = wp_.tile([128, 1], F32, tag="eps")
        nc.vector.memset(eps[:, :], 1e-5)

        # ------------- persistent activation tiles ------------------------
        TOK = BL * F           # 324
        hT = ap_.tile([128, 4, TOK], F16, tag="hT")
        qT = ap_.tile([128, 4, TOK], F16, tag="qT")
        kT = ap_.tile([128, 4, TOK], F16, tag="kT")
        x_b = [ap_.tile([F, C], F32, tag=f"x{b}", name=f"x_b{b}") for b in range(BL)]
        v_b = [ap_.tile([F, C], F16, tag=f"v{b}", name=f"v_b{b}") for b in range(BL)]

        def layer_norm(src, rows, gname, bname, dst):
            """src [rows, C] f32 (partitions 0..rows-1) -> dst [rows, C] f16"""
            st = lp.tile([128, 6], F32, tag="ln_st")
            mv = lp.tile([128, 2], F32, tag="ln_mv")
            rstd = lp.tile([128, 1], F32, tag="ln_rstd")
            xh = lp.tile([128, C], F32, tag="ln_xh")
            nc.vector.bn_stats(out=st[:rows, :], in_=src)
            nc.vector.bn_aggr(out=mv[:rows, :], in_=st[:rows, :])
            nc.scalar.activation(out=rstd[:rows, :], in_=mv[:rows, 1:2],
                                 func=AF.Sqrt, bias=eps[:rows, :], scale=1.0)
            nc.vector.reciprocal(out=rstd[:rows, :], in_=rstd[:rows, :])
            nc.vector.tensor_scalar(out=xh[:rows, :], in0=src,
                                    scalar1=mv[:rows, 0:1],
                                    scalar2=rstd[:rows, :],
                                    op0=AL.subtract, op1=AL.mult)
            nc.vector.tensor_mul(out=xh[:rows, :], in0=xh[:rows, :],
                                 in1=bc[gname][:rows, :])
            nc.vector.tensor_add(out=dst, in0=xh[:rows, :],
                                 in1=bc[bname][:rows, :])

        # ================= stage A: LN1, h, hT ============================
        loop_cm = tc.For_i(0, iters, 1) if iters > 1 else None
        if loop_cm is not None:
            loop_cm.__enter__()
        ppAB = tc.tile_pool(name="ppAB", bufs=2, space="PSUM")
        pp = ppAB.__enter__()
        for b in range(BL):
            nc.sync.dma_start(out=x_b[b][:, :], in_=xc[b])
            h_b = lp.tile([F, C], F16, tag="h_b")
            layer_norm(x_b[b][:, :], F, "n1w", "n1b", h_b[:, :])
            for ch in range(4):
                ps = pp.tile([128, F], F16, tag="tpA")
                nc.tensor.transpose(out=ps[:, :],
                                    in_=h_b[:, ch * 128:(ch + 1) * 128],
                                    identity=ident[0:F, 0:F])
                nc.vector.tensor_copy(out=hT[:, ch, b * F:(b + 1) * F],
                                      in_=ps[:, :])

        # ================= stage B: qT, kT, v =============================
        for nm, dst in (("wq", qT), ("wk", kT)):
            for ch in range(4):
                ps = pp.tile([128, TOK], F32, tag="psQK")
                for kk in range(4):
                    nc.tensor.matmul(out=ps[:, :],
                                     lhsT=wsb[nm][:, kk, ch * 128:(ch + 1) * 128],
                                     rhs=hT[:, kk, :],
                                     start=(kk == 0), stop=(kk == 3))
                nc.vector.tensor_copy(out=dst[:, ch, :], in_=ps[:, :])
        for b in range(BL):
            ps = pp.tile([F, C], F32, tag="psV")
            for kk in range(4):
                nc.tensor.matmul(out=ps[:, :],
                                 lhsT=hT[:, kk, b * F:(b + 1) * F],
                                 rhs=wsb["wv"][:, kk, :],
                                 start=(kk == 0), stop=(kk == 3))
            nc.vector.tensor_copy(out=v_b[b][:, :], in_=ps[:, :])

        ppAB.__exit__(None, None, None)
        # ================= stage C: attention =============================
        ppC = tc.tile_pool(name="ppC", bufs=2, space="PSUM")
        pp = ppC.__enter__()
        NG = 9           # i-group size; 81 = 9 groups of 9
        NT = NG * F      # 729 strip length
        for b in range(BL):
            scoreM = lp.tile([F, F], F32, tag="scoreM")
            for g in range(9):
                i0 = g * NG
                Tb = lp.tile([128, 4, NG, F], F16, tag="Tb")
                for ch in range(4):
                    ks = kT[:, ch, b * F:(b + 1) * F]
                    kb = bass.AP(tensor=ks.tensor, offset=ks.offset,
                                 ap=[ks.ap[0], [0, NG], ks.ap[-1]])
                    qs = qT[:, ch, b * F + i0:b * F + i0 + NG]
                    qb = bass.AP(tensor=qs.tensor, offset=qs.offset,
                                 ap=[qs.ap[0], qs.ap[-1], [0, F]])
                    nc.vector.scalar_tensor_tensor(
                        out=Tb[:, ch, :, :], in0=kb, scalar=0.0, in1=qb,
                        op0=AL.add, op1=AL.add)
                # relu: chunk 0 on DVE, chunks 1-3 on ACT
                nc.vector.tensor_scalar(out=Tb[:, 0, :, :],
                                        in0=Tb[:, 0, :, :],
                                        scalar1=0.0, scalar2=None,
                                        op0=AL.max)
                nc.scalar.activation(out=Tb[:, 1:4, :, :],
                                     in_=Tb[:, 1:4, :, :], func=AF.Relu)
                ps = pp.tile([1, NT], F32, tag="psS")
                for (o, n) in ((0, 512), (512, NT - 512)):
                    for ch in range(4):
                        nc.tensor.matmul(
                            out=ps[0:1, o:o + n],
                            lhsT=ones[:, :],
                            rhs=Tb[:, ch].rearrange("p a b -> p (a b)")[:, o:o + n],
                            start=(ch == 0), stop=(ch == 3))
                sc = lp.tile([1, NT], F32, tag="sc")
                if g % 2 == 0:
                    nc.vector.tensor_copy(out=sc[:, :], in_=ps[0:1, :])
                else:
                    nc.scalar.copy(out=sc[:, :], in_=ps[0:1, :])
                nc.sync.dma_start(out=scoreM[i0:i0 + NG, :], in_=sc[:, :])
            # softmax
            mrow = lp.tile([F, 1], F32, tag="mrow")
            nmrow = lp.tile([F, 1], F32, tag="nmrow")
            sexp = lp.tile([F, 1], F32, tag="sexp")
            attn = lp.tile([F, F], F16, tag="attn")
            attnT = lp.tile([F, F], F16, tag="attnT")
            nc.vector.reduce_max(out=mrow[:, :], in_=scoreM[:, :], axis=AX.X)
            nc.vector.tensor_scalar(out=nmrow[:, :], in0=mrow[:, :],
                                    scalar1=-1.0, scalar2=None, op0=AL.mult)
            nc.scalar.activation(out=attn[:, :], in_=scoreM[:, :], func=AF.Exp,
                                 bias=nmrow[:, :], scale=1.0,
                                 accum_out=sexp[:, :])
            nc.vector.reciprocal(out=sexp[:, :], in_=sexp[:, :])
            nc.vector.tensor_scalar(out=attn[:, :], in0=attn[:, :],
                                    scalar1=sexp[:, :], scalar2=None,
                                    op0=AL.mult)
            psT = pp.tile([F, F], F16, tag="c_small", bufs=3, name="psT")
            nc.tensor.transpose(out=psT[:, :], in_=attn[:, :],
                                identity=ident[0:F, 0:F])
            nc.vector.tensor_copy(out=attnT[:, :], in_=psT[:, :])
            aTs = lp.tile([128, 4, F], F16, tag="aTs")
            for ch in range(4):
                psV = pp.tile([128, F], F32, tag="c_small", bufs=3, name="psV")
                nc.tensor.matmul(out=psV[:, :],
                                 lhsT=v_b[b][:, ch * 128:(ch + 1) * 128],
                                 rhs=attnT[:, :], start=True, stop=True)
                nc.vector.tensor_copy(out=aTs[:, ch, :], in_=psV[:, :])
            psP = pp.tile([F, C], F32, tag="c_small", bufs=3, name="psP")
            for ch in range(4):
                nc.tensor.matmul(out=psP[:, :], lhsT=aTs[:, ch, :],
                                 rhs=wsb["wp"][:, ch, :],
                                 start=(ch == 0), stop=(ch == 3))
            nc.vector.scalar_tensor_tensor(out=x_b[b][:, :], in0=psP[:, :],
                                           scalar=0.0, in1=x_b[b][:, :],
                                           op0=AL.bypass, op1=AL.add)
            nc.vector.tensor_add(out=x_b[b][:, :], in0=x_b[b][:, :],
                                 in1=bc["bp"][0:F, :])

        ppC.__exit__(None, None, None)
        # ================= stage D: branch 1 (rows 0:40) ==================
        ppD = tc.tile_pool(name="ppD", bufs=2, space="PSUM")
        pp = ppD.__enter__()
        h2T = ap_.tile([128, 4, BL * F1H], F16, tag="h2T")
        for b in range(BL):
            h2_b = lp.tile([F1H, C], F16, tag="h2_b")
            layer_norm(x_b[b][0:F1H, :], F1H, "n2w", "n2b", h2_b[:, :])
            for ch in range(4):
                ps = pp.tile([128, F1H], F16, tag="tpD")
                nc.tensor.transpose(out=ps[:, :],
                                    in_=h2_b[:, ch * 128:(ch + 1) * 128],
                                    identity=ident[0:F1H, 0:F1H])
                nc.vector.tensor_copy(
                    out=h2T[:, ch, b * F1H:(b + 1) * F1H], in_=ps[:, :])
        xa_p0 = ap_.tile([128, C], F32, tag="xa_p0")
        xa_p1 = ap_.tile([32, C], F32, tag="xa_p1")
        for b in range(BL):
            r0 = b * F1H
            if r0 + F1H <= 128:
                nc.sync.dma_start(out=xa_p0[r0:r0 + F1H, :],
                                  in_=x_b[b][0:F1H, :])
            else:
                nc.sync.dma_start(out=xa_p0[r0:128, :],
                                  in_=x_b[b][0:128 - r0, :])
                nc.sync.dma_start(out=xa_p1[0:r0 + F1H - 128, :],
                                  in_=x_b[b][128 - r0:F1H, :])
        g_sb = ap_.tile([128, 16, BL * F2H], F16, tag="g")  # sized for max use
        TOK1 = BL * F1H
        for hch in range(16):
            ps = pp.tile([128, TOK1], F32, tag="psF1")
            for kk in range(4):
                nc.tensor.matmul(out=ps[:, :],
                                 lhsT=w11[:, kk, hch * 128:(hch + 1) * 128],
                                 rhs=h2T[:, kk, :],
                                 start=(kk == 0), stop=(kk == 3))
            nc.scalar.activation(out=g_sb[:, hch, 0:TOK1], in_=ps[:, :],
                                 func=AF.Gelu, bias=b11[:, hch:hch + 1],
                                 scale=1.0)
        for (tsl, tn) in ((0, 128), (128, 32)):
            ps = pp.tile([128, C], F32, tag="psF2")
            for hch in range(16):
                nc.tensor.matmul(out=ps[0:tn, :],
                                 lhsT=g_sb[:, hch, tsl:tsl + tn],
                                 rhs=w12[:, hch, :],
                                 start=(hch == 0), stop=(hch == 15))
            o1 = lp.tile([128, C], F32, tag="o1")
            xa_src = xa_p0[tsl:tsl + tn, :] if tsl == 0 else xa_p1[0:tn, :]
            nc.vector.scalar_tensor_tensor(out=o1[0:tn, :], in0=ps[0:tn, :],
                                           scalar=0.0, in1=xa_src,
                                           op0=AL.bypass, op1=AL.add)
            nc.vector.tensor_add(out=o1[0:tn, :], in0=o1[0:tn, :],
                                 in1=bc["b12"][0:tn, :])
            for b in range(BL):
                r0, r1 = b * F1H, (b + 1) * F1H
                lo, hi = max(r0, tsl), min(r1, tsl + tn)
                if lo < hi:
                    nc.sync.dma_start(
                        out=yc[b, lo - r0:hi - r0, :],
                        in_=o1[lo - tsl:hi - tsl, :])

        ppD.__exit__(None, None, None)
        # ================= stage E: branch 2 (rows 40:81) =================
        ppE = tc.tile_pool(name="ppE", bufs=2, space="PSUM")
        pp = ppE.__enter__()
        # load branch-2 MLP weights (shared slots unless share_w=False,
        # in which case they were loaded up front)
        if share_w:
            w21 = wp_.tile([128, 4, HID], F16, tag="wA", name="w21")
            nc.sync.dma_start(out=w21[:, :, :],
                              in_=dr["w21"].rearrange("(k p) o -> p k o", p=128))
            w22 = wp_.tile([128, 16, C], F16, tag="wB", name="w22")
            nc.sync.dma_start(out=w22[:, :, :],
                              in_=dr["w22"].rearrange("(k p) o -> p k o", p=128))
        else:
            w21, w22 = w21_pre, w22_pre
        TOK2 = BL * F2H  # 164
        hfT = ap_.tile([128, 4, TOK2], F16, tag="hfT")
        psiT = ap_.tile([128, 4, TOK2], F16, tag="psiT")
        zT = ap_.tile([128, 4, TOK2], F16, tag="zT")
        wT = ap_.tile([128, 4, TOK2], F16, tag="wT")
        xb0 = [ap_.tile([F2H, C], F32, tag=f"xb0_{b}", name=f"xb0_{b}") for b in range(BL)]
        for b in range(BL):
            nc.sync.dma_start(out=xb0[b][:, :], in_=x_b[b][F1H:F, :])
            hf_b = lp.tile([F2H, C], F16, tag="hf_b")
            layer_norm(xb0[b][:, :], F2H, "n3w", "n3b", hf_b[:, :])
            for ch in range(4):
                ps = pp.tile([128, F2H], F16, tag="e_tp", bufs=2, name="psTpE")
                nc.tensor.transpose(out=ps[:, :],
                                    in_=hf_b[:, ch * 128:(ch + 1) * 128],
                                    identity=ident[0:F2H, 0:F2H])
                nc.vector.tensor_copy(
                    out=hfT[:, ch, b * F2H:(b + 1) * F2H], in_=ps[:, :])
            for ch in range(4):
                for nm, dst in (("pft", psiT), ("f1t", zT), ("f2t", wT)):
                    ps = pp.tile([128, TOK2], F32, tag="e_mm", bufs=3, name="psZ")
                    nc.tensor.matmul(out=ps[:, 0:F2H],
                                     lhsT=hf_b[:, ch * 128:(ch + 1) * 128],
                                     rhs=fm[nm][:, :], start=True, stop=True)
                    nc.vector.tensor_copy(
                        out=dst[:, ch, b * F2H:(b + 1) * F2H], in_=ps[:, 0:F2H])
        lowT = ap_.tile([128, 4, TOK2], F16, tag="lowT")
        for ch in range(4):
            ps = pp.tile([128, TOK2], F32, tag="e_mm", bufs=3, name="psL")
            for kk in range(4):
                nc.tensor.matmul(out=ps[:, :],
                                 lhsT=wsb["pc"][:, kk, ch * 128:(ch + 1) * 128],
                                 rhs=psiT[:, kk, :],
                                 start=(kk == 0), stop=(kk == 3))
            nc.vector.tensor_copy(out=lowT[:, ch, :], in_=ps[:, :])
        for hch in range(16):
            ps = pp.tile([128, TOK2], F32, tag="e_mm", bufs=3, name="psF3")
            for kk in range(4):
                nc.tensor.matmul(out=ps[:, :],
                                 lhsT=w21[:, kk, hch * 128:(hch + 1) * 128],
                                 rhs=lowT[:, kk, :],
                                 start=(kk == 0), stop=(kk == 3))
            nc.scalar.activation(out=g_sb[:, hch, 0:TOK2], in_=ps[:, :],
                                 func=AF.Gelu, bias=b21[:, hch:hch + 1],
                                 scale=1.0)
        uT = ap_.tile([128, 4, TOK2], F16, tag="uT")
        for och in range(4):
            ps = pp.tile([128, TOK2], F32, tag="e_mm", bufs=3, name="psF4")
            for hch in range(16):
                nc.tensor.matmul(out=ps[:, :],
                                 lhsT=w22[:, hch, och * 128:(och + 1) * 128],
                                 rhs=g_sb[:, hch, 0:TOK2],
                                 start=(hch == 0), stop=(hch == 15))
            nc.scalar.activation(out=uT[:, och, :], in_=ps[:, :],
                                 func=AF.Identity,
                                 bias=b22pp[:, och:och + 1], scale=1.0)
        t1T = ap_.tile([128, 4, TOK2], F16, tag="t1T")
        for o2 in range(4):
            ps = pp.tile([128, TOK2], F32, tag="e_mm", bufs=3, name="psT1")
            for och in range(4):
                nc.tensor.matmul(out=ps[:, :],
                                 lhsT=wsb["gc"][:, och, o2 * 128:(o2 + 1) * 128],
                                 rhs=uT[:, och, :],
                                 start=(och == 0), stop=(och == 3))
            nc.vector.tensor_copy(out=t1T[:, o2, :], in_=ps[:, :])
        for b in range(BL):
            t1_b = lp.tile([F2H, 4, 128], F16, tag="t1_b")
            for ch in range(4):
                ps = pp.tile([F2H, 128], F16, tag="e_tp", bufs=2, name="psTpT1")
                nc.tensor.transpose(out=ps[:, :],
                                    in_=t1T[:, ch, b * F2H:(b + 1) * F2H],
                                    identity=ident[:, :])
                nc.vector.tensor_copy(out=t1_b[:, ch, :], in_=ps[:, :])
            psR = pp.tile([F2H, C], F32, tag="psR", bufs=2)
            nc.tensor.matmul(out=psR[:, :], lhsT=fm["f0t"][:, :],
                             rhs=t1_b.rearrange("p a b -> p (a b)"),
                             start=True, stop=False)
            for ch in range(4):
                nc.tensor.matmul(out=psR[:, :],
                                 lhsT=zT[:, ch, b * F2H:(b + 1) * F2H],
                                 rhs=wsb["n1m"][:, ch, :],
                                 start=False, stop=False)
            for ch in range(4):
                nc.tensor.matmul(out=psR[:, :],
                                 lhsT=wT[:, ch, b * F2H:(b + 1) * F2H],
                                 rhs=wsb["n2m"][:, ch, :],
                                 start=False, stop=(ch == 3))
            nc.vector.scalar_tensor_tensor(out=xb0[b][:, :], in0=psR[:, :],
                                           scalar=0.0, in1=xb0[b][:, :],
                                           op0=AL.bypass, op1=AL.add)
            nc.sync.dma_start(out=yc[b, F1H:F, :], in_=xb0[b][:, :])
        ppE.__exit__(None, None, None)
        if loop_cm is not None:
            loop_cm.__exit__(None, None, None)

    nc.compile()
    return nc


def _prep_inputs(inputs):
    """Full inputs -> (common weight map, list of per-core maps)."""
    ins = {k: np.asarray(v) for k, v in inputs.items()}
    mats = host_mats()
    com = {}
    for nm in ("wq", "wk", "wv", "wp", "w11", "w12", "w21", "w22"):
        com[nm] = ins[nm].astype(np.float16)
    for nm, arr in mats.items():
        com[nm] = arr
    for src, dst in (("bp", "bp"), ("b11", "b11"), ("b12", "b12"),
                     ("b21", "b21"), ("b22", "b22"),
                     ("norm1_w", "n1w"), ("norm1_b", "n1b"),
                     ("norm2_w", "n2w"), ("norm2_b", "n2b"),
                     ("norm3_w", "n3w"), ("norm3_b", "n3b")):
        com[dst] = ins[src].astype(np.float32)
    x = ins["x"].astype(np.float32)
    in_maps = []
    for c in range(NCORES):
        m = dict(com)
        m["xc"] = np.ascontiguousarray(x[c * BL:(c + 1) * BL])
        in_maps.append(m)
    return in_maps


def kernel(**inputs):
    from concourse.bass_utils import run_bass_kernel_spmd
    if "nc" not in _CACHE:
        _CACHE["nc"] = build_nc()
    nc = _CACHE["nc"]
    in_maps = _prep_inputs(inputs)
    res = run_bass_kernel_spmd(nc, in_maps, core_ids=list(range(NCORES)))
    out = np.concatenate([res.results[c]["yc"] for c in range(NCORES)], axis=0)
    return out.astype(np.float32)
